# revision 8
# baseline (speedup 1.0000x reference)
"""Trainium2 Bass kernel for CausalSelfAttention (B=2, T=4096, C=1024, 16 heads, RoPE).

Sharding: tensor-parallel across heads. Core c handles heads {2c, 2c+1} for
both batches; the two batches are processed as two "units".

Per core (v3 — tightened head/tail schedule):
  - QKV emitted as pieces drained into the attention stream (PE never idles):
    qT/kT computed transposed ([dims, T]), v natural. RoPE via partition-
    rotated copy (sign folded into the bf16 sin table). cos/sin tables are
    loaded at 32/64 distinct rows and partition-replicated on-chip.
  - Attention paces ScalarE: per 128-k chunk, a row-tiled score matmul pair
    (both heads on PE row tiles) into a [128, 2, 512] PSUM strip, exp on
    ScalarE, triangle mask on diagonal chunks, K=128 AV matmuls accumulating
    y + softmax denominator (65th ones column) into a [128, 2, 512] PSUM
    accumulator.
  - Epilogue: reciprocal straight off the PSUM denominator row, SBUF->SBUF
    partition broadcast, one fused normalize multiply into yT2, two
    a2a-staging DMAs (split across queues).
  - Eight per-quarter 8-way AllToAlls (one per 1024 tokens of a unit,
    y^T head-sharded -> token-sharded); proj m-tiles run in the background
    stream two collectives behind, so only the last quarter's collective and
    one proj tile are exposed at the tail.

PSUM budget (8 banks): 2x2 score strips + 2 AV accumulator + 2 qkv/proj ring.
kernel() takes the full unsharded inputs and returns the full output.
"""

import numpy as np
import ml_dtypes

import concourse.bass as bass
import concourse.bacc as bacc
import concourse.mybir as mybir
import concourse.tile as tile

BF16 = mybir.dt.bfloat16
F32 = mybir.dt.float32
NPBF16 = ml_dtypes.bfloat16

N_EMBD = 1024
N_HEAD = 16
HS = 64
B = 2
T_FULL = 4096
QT = 512            # q-tile width
KTILE = 128         # k positions per chunk
N_CORES = 8

AluAdd = mybir.AluOpType.add
AluMult = mybir.AluOpType.mult


def build_nc(T=T_FULL):
    assert T % QT == 0
    NQT = T // QT          # q-tiles per unit (= per batch)
    NT = T // KTILE        # 128-wide k tiles per unit
    NQTR = NQT // 2        # quarters (j-pairs) per unit
    nc = bacc.Bacc()

    xT_d = nc.declare_dram_parameter("xT", [B * T // QT, 128, 8, QT], BF16, isOutput=False)
    wq_d = nc.declare_dram_parameter("Wq", [128, 8, 128], BF16, isOutput=False)
    wk_d = nc.declare_dram_parameter("Wk", [128, 8, 128], BF16, isOutput=False)
    wv_d = nc.declare_dram_parameter("Wv", [128, 8, 128], BF16, isOutput=False)
    bq_d = nc.declare_dram_parameter("bq", [128, 2], F32, isOutput=False)
    bk_d = nc.declare_dram_parameter("bk", [128, 2], F32, isOutput=False)
    bv_d = nc.declare_dram_parameter("bv_bc", [128, 128], F32, isOutput=False)
    cos_d = nc.declare_dram_parameter("cosT", [32, T], BF16, isOutput=False)
    sin_d = nc.declare_dram_parameter("sinT", [64, T], BF16, isOutput=False)
    tri_d = nc.declare_dram_parameter("tri", [128, 2, 128], BF16, isOutput=False)
    wp_d = nc.declare_dram_parameter("Wp", [128, 8, N_EMBD], BF16, isOutput=False)
    bp_d = nc.declare_dram_parameter("bp_bc", [128, N_EMBD], BF16, isOutput=False)
    # out rows: [unit, quarter, 128 tokens of this core's block, N_EMBD]
    out_d = nc.declare_dram_parameter("out", [B, NQTR, 128, N_EMBD], F32, isOutput=True)

    rec_d = nc.dram_tensor("rec_scratch", [B, NQT, 2, QT], F32)
    sync_in = nc.dram_tensor("sync_in", [8, 1, 16], BF16)
    sync_out = nc.dram_tensor("sync_out", [8, 1, 16], BF16)
    # per-quarter all-to-all staging: [slot, 2 heads x 64 dims, 128 tokens]
    a2a_in = [[nc.dram_tensor(f"a2a_in{u}{q}", [8, 128, 128], BF16) for q in range(NQTR)]
              for u in range(B)]
    a2a_out = [[nc.dram_tensor(f"a2a_out{u}{q}", [8, 128, 128], BF16) for q in range(NQTR)]
               for u in range(B)]

    with tile.TileContext(nc) as tc, \
         tc.tile_pool(name="const", bufs=1) as const, \
         tc.tile_pool(name="persist", bufs=1) as persist, \
         tc.tile_pool(name="xc", bufs=4) as xpool, \
         tc.tile_pool(name="stage", bufs=3) as stage, \
         tc.tile_pool(name="ppool", bufs=3) as ppool, \
         tc.tile_pool(name="epi", bufs=2) as epi, \
         tc.tile_pool(name="projp", bufs=2) as projp, \
         tc.tile_pool(name="qkv_ps", bufs=2, space="PSUM") as qkv_ps, \
         tc.tile_pool(name="strip_ps", bufs=2, space="PSUM") as strip_ps, \
         tc.tile_pool(name="av_ps", bufs=1, space="PSUM") as av_ps:

        # ---- constants.  Scalar queue: weights (in first-use order).
        # Vector queue: cos/sin loads + on-chip partition replication.
        # Sync queue: xc chunk DMAs.  GpSimd queue: collectives only. ----
        wq_sb = const.tile([128, 8, 128], BF16, tag="wq")
        wk_sb = const.tile([128, 8, 128], BF16, tag="wk")
        wv_sb = const.tile([128, 8, 128], BF16, tag="wv")
        bq_sb = const.tile([128, 2], F32, tag="bq")
        bk_sb = const.tile([128, 2], F32, tag="bk")
        bv_sb = const.tile([128, 128], F32, tag="bv")
        tri_sb = const.tile([128, 2, 128], BF16, tag="tri")
        nc.scalar.dma_start(out=wq_sb[:], in_=wq_d[:])
        nc.scalar.dma_start(out=bq_sb[:], in_=bq_d[:])
        nc.scalar.dma_start(out=wk_sb[:], in_=wk_d[:])
        nc.scalar.dma_start(out=bk_sb[:], in_=bk_d[:])
        nc.scalar.dma_start(out=wv_sb[:], in_=wv_d[:])
        nc.scalar.dma_start(out=bv_sb[:], in_=bv_d[:])
        nc.scalar.dma_start(out=tri_sb[:], in_=tri_d[:])

        cos_sb = const.tile([128, T], BF16, tag="cos")
        sin_sb = const.tile([128, T], BF16, tag="sin")
        nc.gpsimd.dma_start(out=cos_sb[0:32, :], in_=cos_d[:])
        nc.gpsimd.dma_start(out=sin_sb[0:64, :], in_=sin_d[:])
        nc.gpsimd.dma_start(out=cos_sb[32:64, :], in_=cos_sb[0:32, :])
        nc.gpsimd.dma_start(out=cos_sb[64:128, :], in_=cos_sb[0:64, :])
        nc.gpsimd.dma_start(out=sin_sb[64:128, :], in_=sin_sb[0:64, :])

        # proj weights load later (background piece) to keep boot HBM free
        wp_sb = const.tile([128, 8, N_EMBD], BF16, tag="wp")
        bp_sb = const.tile([128, N_EMBD], BF16, tag="bp")

        def load_wp():
            nc.scalar.dma_start(out=wp_sb[:], in_=wp_d[:])
            nc.scalar.dma_start(out=bp_sb[:], in_=bp_d[:])

        # ---- persistent per-unit tensors ----
        qT = [persist.tile([128, T], BF16, tag=f"qT{u}", name=f"qT{u}") for u in range(B)]
        kT = [persist.tile([128, T], BF16, tag=f"kT{u}", name=f"kT{u}") for u in range(B)]
        vP = [persist.tile([128, NT, 130], BF16, tag=f"vP{u}", name=f"vP{u}") for u in range(B)]
        # normalized y^T: [64 dims, head, T]
        yT2 = [persist.tile([64, 2, T], BF16, tag=f"yT{u}", name=f"yT{u}") for u in range(B)]
        for u in range(B):
            nc.vector.memset(vP[u][:, :, 64:65], 1.0)
            nc.vector.memset(vP[u][:, :, 129:130], 1.0)
        # tiny warmup all-to-all: absorbs boot-time core skew during the idle
        # head so the first real collective doesn't stall mid-attention
        nc.gpsimd.collective_compute(
            "AllToAll", mybir.AluOpType.bypass,
            replica_groups=[[0, 1, 2, 3, 4, 5, 6, 7]],
            ins=[sync_in[:]], outs=[sync_out[:]],
        )

        def qkv_chunk_pieces(u, ch):
            """Pieces (closures) computing qT/kT/vP for tokens [ch*512,(ch+1)*512) of unit u."""
            st = {}

            def p_xc():
                chg = u * (T // QT) + ch
                xc = xpool.tile([128, 8, QT], BF16, tag="xc", name="xc")
                nc.sync.dma_start(out=xc[:], in_=xT_d[chg])
                st["xc"] = xc

            def mk_qk(w_sb, b_sb, dstT):
                def p_qk():
                    csl = slice(ch * QT, (ch + 1) * QT)
                    xc = st["xc"]
                    pA = qkv_ps.tile([128, QT], F32, tag="qk", name="pA")
                    for ct in range(8):
                        nc.tensor.matmul(pA[:], w_sb[:, ct, :], xc[:, ct, :],
                                         start=(ct == 0), stop=(ct == 7))
                    qa = stage.tile([128, QT], BF16, tag="qa", name="qa", bufs=3)
                    nc.vector.tensor_scalar_add(qa[:], pA[:], b_sb[:, 0:1])
                    qr = stage.tile([128, QT], BF16, tag="qr", name="qr", bufs=3)
                    for (dp, sp) in ((0, 32), (32, 0), (64, 96), (96, 64)):
                        nc.sync.dma_start(out=qr[dp:dp + 32, :], in_=qa[sp:sp + 32, :])
                    m1 = stage.tile([128, QT], BF16, tag="m1", name="m1", bufs=2)
                    m2 = stage.tile([128, QT], BF16, tag="m2", name="m2", bufs=2)
                    nc.vector.tensor_mul(m1[:], qa[:], cos_sb[:, csl])
                    nc.vector.tensor_mul(m2[:], qr[:], sin_sb[:, csl])
                    nc.vector.tensor_add(dstT[:, csl], m1[:], m2[:])
                return p_qk

            def mk_v(t4):
                def p_v():
                    xc = st["xc"]
                    ttg = ch * 4 + t4
                    tsl = slice(t4 * 128, (t4 + 1) * 128)
                    pV = qkv_ps.tile([128, QT], F32, tag="qk", name="pV")
                    for ct in range(8):
                        nc.tensor.matmul(pV[:, 0:128], xc[:, ct, tsl], wv_sb[:, ct, :],
                                         start=(ct == 0), stop=(ct == 7))
                    pv2 = bass.AP(tensor=pV.tensor, offset=pV.offset,
                                  ap=[list(pV.ap[0]), [64, 2], [1, 64]])
                    dst = vP[u][:, ttg, 0:129]
                    dst2 = bass.AP(tensor=dst.tensor, offset=dst.offset,
                                   ap=[list(dst.ap[0]), [65, 2], [1, 64]])
                    bv2 = bass.AP(tensor=bv_sb.tensor, offset=bv_sb.offset,
                                  ap=[list(bv_sb.ap[0]), [64, 2], [1, 64]])
                    nc.vector.tensor_add(dst2, pv2, bv2)
                return p_v

            return [p_xc, mk_qk(wq_sb, bq_sb, qT[u]), mk_qk(wk_sb, bk_sb, kT[u])] + \
                [mk_v(t4) for t4 in range(4)]

        def attn_block(u, j, drain):
            """Attention for q-tile j (512 q) of unit u against k tiles 0..4j+3.
            Calls drain() between chunks to interleave background PE work."""
            jsl = slice(j * QT, (j + 1) * QT)
            nchunks = 4 * (j + 1)
            av_t = av_ps.tile([128, 2, QT], F32, tag="av", name="av_t")

            def av_emit(pend):
                P, c, qoff, w = pend
                first, last = (c == 0), (c == nchunks - 1)
                for h in range(2):
                    nc.tensor.matmul(av_t[0:65, h, qoff:QT],
                                     vP[u][:, c, 65 * h:65 * h + 65],
                                     P[:, h, 0:w],
                                     start=first, stop=last)

            pend = None
            for c in range(nchunks):
                s = c - (nchunks - 4)          # diagonal sub-position 0..3, or <0
                qoff = 128 * s if s > 0 else 0
                w = QT - qoff
                ksl = slice(c * KTILE, (c + 1) * KTILE)
                strip = strip_ps.tile([128, 2, QT], F32, tag="strip", name="strip")
                for h in range(2):
                    hsl = slice(64 * h, 64 * (h + 1))
                    nc.tensor.matmul(strip[:, h, 0:w], kT[u][hsl, ksl],
                                     qT[u][hsl, j * QT + qoff:(j + 1) * QT],
                                     start=True, stop=True)
                P = ppool.tile([128, 2, QT], BF16, tag="P", name="P", bufs=4)
                nc.scalar.activation(P[:, :, 0:w], strip[:, :, 0:w],
                                     mybir.ActivationFunctionType.Exp)
                if s >= 0:  # leading 128 cols of a diagonal chunk: triangle mask
                    nc.vector.tensor_mul(P[:, :, 0:128], P[:, :, 0:128], tri_sb[:])
                if pend is not None:
                    av_emit(pend)
                pend = (P, c, qoff, w)
                if c % 3 == 2:
                    drain(1)
            av_emit(pend)
            # epilogue: evacuate y + denominator (frees the PSUM accumulator),
            # reciprocal, DRAM-bounce partition broadcast, normalize.
            yc = epi.tile([64, 2, QT], BF16, tag="yc", name="yc")
            nc.vector.tensor_copy(yc[:], av_t[0:64, :, :])
            den = epi.tile([1, 2, QT], F32, tag="den", name="den")
            nc.vector.tensor_copy(den[:], av_t[64:65, :, :])
            rc = epi.tile([1, 2, QT], F32, tag="rc", name="rc")
            nc.vector.reciprocal_approx_fast(rc[0:1, :, :], den[0:1, :, :])
            rb = epi.tile([64, 2, QT], F32, tag="rb", name="rb", bufs=1)
            nc.sync.dma_start(out=rec_d[u, j], in_=rc[0:1, :, :])
            dsrc = rec_d[u, j]
            bsrc = bass.AP(tensor=dsrc.tensor, offset=dsrc.offset,
                           ap=[[0, 64]] + list(dsrc.ap))
            nc.sync.dma_start(out=rb[:], in_=bsrc)
            nc.vector.tensor_mul(yT2[u][:, :, jsl], yc[:], rb[:])
            # stage into the quarter's all-to-all buffer:
            # a2a_in[u][q][s, 64h+r, t] = yT2[u][r, h, j*512 + (s-4*(j%2))*128 + t]
            q2, jh = j // 2, j % 2
            t_in = a2a_in[u][q2][:]
            for h in range(2):
                dst = bass.AP(tensor=t_in.tensor,
                              offset=t_in.offset + jh * 4 * 16384 + h * 8192,
                              ap=[[128, 64], [16384, 4], [1, 128]])
                qdma = nc.sync.dma_start if h == 0 else nc.scalar.dma_start
                qdma(out=dst, in_=yT2[u][:, h, jsl])

        def a2a_start(u, q):
            nc.gpsimd.collective_compute(
                "AllToAll", mybir.AluOpType.bypass,
                replica_groups=[[0, 1, 2, 3, 4, 5, 6, 7]],
                ins=[a2a_in[u][q][:]], outs=[a2a_out[u][q][:]],
            )

        def proj_mtile(u, q):
            """out rows for this core's 128-token block of quarter q of unit u."""
            ydm = projp.tile([128, 8, 128], BF16, tag="ydm", name="ydm")
            t_out = a2a_out[u][q][:]
            # ydm[d, s, t] = a2a_out[u][q][s, d, t]  (one DMA)
            src3 = bass.AP(tensor=t_out.tensor, offset=t_out.offset,
                           ap=[[128, 128], [16384, 8], [1, 128]])
            nc.scalar.dma_start(out=ydm[:], in_=src3)
            ob = projp.tile([128, N_EMBD], F32, tag="ob", name="ob", bufs=1)
            for nh2 in range(2):
                nsl = slice(nh2 * 512, (nh2 + 1) * 512)
                pp = qkv_ps.tile([128, QT], F32, tag="qk", name="pp")
                for ft in range(8):
                    nc.tensor.matmul(pp[:], ydm[:, ft, :], wp_sb[:, ft, nsl],
                                     start=(ft == 0), stop=(ft == 7))
                nc.vector.tensor_add(ob[:, nsl], pp[:], bp_sb[:, nsl])
            nc.sync.dma_start(out=out_d[u, q], in_=ob[:])

        # ---- schedule: attention paces ScalarE; qkv/proj pieces fill PE gaps ----
        bg = []          # list of (key, piece_fn); key=(u, ch) for qkv, None otherwise
        bgi = [0]
        qkv_done = {}    # u -> highest chunk fully emitted

        def drain(n):
            for _ in range(n):
                if bgi[0] >= len(bg):
                    return
                key, fn = bg[bgi[0]]
                bgi[0] += 1
                fn()
                if key is not None:
                    qkv_done[key[0]] = key[1]

        def add_chunk(u, ch):
            ps = qkv_chunk_pieces(u, ch)
            # key only on the LAST piece: chunk counts as emitted when all pieces ran
            bg.extend((None, p) for p in ps[:-1])
            bg.append(((u, ch), ps[-1]))

        ps0 = qkv_chunk_pieces(0, 0)
        for p in ps0:
            p()
        qkv_done[0] = 0
        add_chunk(0, 1)
        bg.append((None, load_wp))
        for ch in range(2, NQT):
            add_chunk(0, ch)
        for ch in range(NQT):
            add_chunk(1, ch)

        for u in range(B):
            for j in range(NQT):
                while qkv_done.get(u, -1) < j:
                    drain(1)
                attn_block(u, j, drain)
                if j % 2 == 1:
                    q = j // 2
                    a2a_start(u, q)
                    k = NQTR * u + q
                    if k >= 2:
                        pu, pq = divmod(k - 2, NQTR)
                        bg.append((None, lambda pu=pu, pq=pq: proj_mtile(pu, pq)))
        while bgi[0] < len(bg):
            drain(1)
        # the last two quarters' proj fills the final collective's latency
        for k in range(max(0, 2 * NQTR - 2), 2 * NQTR):
            pu, pq = divmod(k, NQTR)
            proj_mtile(pu, pq)

    nc.compile()
    return nc


def make_inputs(x, W_attn, b_attn, W_proj, b_proj, T):
    """Build the 8 per-core input maps from full inputs."""
    scale = 1.0 / np.sqrt(HS)
    inv_freq = 1.0 / (10000.0 ** (np.arange(0, HS, 2, dtype=np.float64) / HS))  # [32]
    t = np.arange(T, dtype=np.float64)
    freqs = np.outer(t, inv_freq)  # [T, 32]
    cos32 = np.cos(freqs).T.astype(np.float32)               # [32, T]
    sin32 = np.sin(freqs).T.astype(np.float32)               # [32, T]
    sin64 = np.concatenate([-sin32, sin32], axis=0)          # [64, T]

    # triangle mask for the leading 128 cols of diagonal chunks: 1 iff p <= f
    p = np.arange(128)[:, None]
    f = np.arange(128)[None, :]
    tri = (p <= f).astype(np.float32)
    tri2 = np.stack([tri, tri], axis=1)  # [128, 2, 128]

    C = N_EMBD
    # chunk-major x: xh[ch, p, a, t] = x[ch*QT + t, a*128 + p]
    xh = np.ascontiguousarray(
        x.reshape(B * T // QT, QT, 8, 128).transpose(0, 3, 2, 1)).astype(NPBF16)
    # rot permutation of head dims: d -> d+32 (first half) / d-32 (second half)
    d = np.arange(128)
    perm = np.where((d % 64) < 32, d + 32, d - 32)
    bp_bc = np.broadcast_to(b_proj[None, :], (128, N_EMBD)).astype(np.float32).copy()
    in_maps = []
    for c in range(N_CORES):
        hsl = slice(128 * c, 128 * (c + 1))  # dims of heads {2c, 2c+1}
        Wq = W_attn[:, 0 * C:1 * C][:, hsl] * scale
        Wk = W_attn[:, 1 * C:2 * C][:, hsl]
        Wv = W_attn[:, 2 * C:3 * C][:, hsl]
        bq = (b_attn[0 * C:1 * C][hsl] * scale).astype(np.float32)
        bk = b_attn[1 * C:2 * C][hsl].astype(np.float32)
        bv = b_attn[2 * C:3 * C][hsl]
        in_maps.append({
            "xT": xh,
            "Wq": np.ascontiguousarray(Wq.reshape(8, 128, 128).transpose(1, 0, 2)).astype(NPBF16),
            "Wk": np.ascontiguousarray(Wk.reshape(8, 128, 128).transpose(1, 0, 2)).astype(NPBF16),
            "Wv": np.ascontiguousarray(Wv.reshape(8, 128, 128).transpose(1, 0, 2)).astype(NPBF16),
            "bq": np.stack([bq, bq[perm]], axis=1).copy(),
            "bk": np.stack([bk, bk[perm]], axis=1).copy(),
            "bv_bc": np.broadcast_to(bv[None, :], (128, 128)).astype(np.float32).copy(),
            "cosT": cos32.astype(NPBF16),
            "sinT": sin64.astype(NPBF16),
            "tri": tri2.astype(NPBF16),
            "Wp": np.ascontiguousarray(W_proj.reshape(8, 128, N_EMBD).transpose(1, 0, 2)).astype(NPBF16),
            "bp_bc": bp_bc.astype(NPBF16),
        })
    return in_maps


def assemble(results, T):
    NQTR = T // QT // 2
    out = np.empty((B, T, N_EMBD), dtype=np.float32)
    for c in range(N_CORES):
        blk = results[c]["out"]  # [B, NQTR, 128, N_EMBD]
        for u in range(B):
            for q in range(NQTR):
                out[u, q * 1024 + c * 128:q * 1024 + (c + 1) * 128, :] = blk[u, q]
    return out


_NC_CACHE = {}


def kernel(x, W_attn, b_attn, W_proj, b_proj):
    from concourse.bass_utils import run_bass_kernel_spmd
    x = np.asarray(x, dtype=np.float32)
    W_attn = np.asarray(W_attn, dtype=np.float32)
    b_attn = np.asarray(b_attn, dtype=np.float32)
    W_proj = np.asarray(W_proj, dtype=np.float32)
    b_proj = np.asarray(b_proj, dtype=np.float32)
    T = x.shape[1]
    if T not in _NC_CACHE:
        _NC_CACHE[T] = build_nc(T)
    nc = _NC_CACHE[T]
    in_maps = make_inputs(x, W_attn, b_attn, W_proj, b_proj, T)
    res = run_bass_kernel_spmd(nc, in_maps, core_ids=list(range(N_CORES)))
    return assemble(res.results, T)


# revision 18
# speedup vs baseline: 1.0510x; 1.0510x over previous
"""Trainium2 Bass kernel for CausalSelfAttention (B=2, T=4096, C=1024, 16 heads, RoPE).

Sharding: tensor-parallel across heads. Core c handles heads {2c, 2c+1} for
both batches; the two batches are processed as two "units".

Per core (v3 — tightened head/tail schedule):
  - QKV emitted as pieces drained into the attention stream (PE never idles):
    qT/kT computed transposed ([dims, T]), v natural. RoPE via partition-
    rotated copy (sign folded into the bf16 sin table). cos/sin tables are
    loaded at 32/64 distinct rows and partition-replicated on-chip.
  - Attention paces ScalarE: per 128-k chunk, a row-tiled score matmul pair
    (both heads on PE row tiles) into a [128, 2, 512] PSUM strip, exp on
    ScalarE, triangle mask on diagonal chunks, K=128 AV matmuls accumulating
    y + softmax denominator (65th ones column) into a [128, 2, 512] PSUM
    accumulator.
  - Epilogue: reciprocal straight off the PSUM denominator row, SBUF->SBUF
    partition broadcast, one fused normalize multiply into yT2, two
    a2a-staging DMAs (split across queues).
  - Eight per-quarter 8-way AllToAlls (one per 1024 tokens of a unit,
    y^T head-sharded -> token-sharded); proj m-tiles run in the background
    stream two collectives behind, so only the last quarter's collective and
    one proj tile are exposed at the tail.

PSUM budget (8 banks): 2x2 score strips + 2 AV accumulator + 2 qkv/proj ring.
kernel() takes the full unsharded inputs and returns the full output.
"""

import numpy as np
import ml_dtypes

import concourse.bass as bass
import concourse.bacc as bacc
import concourse.mybir as mybir
import concourse.tile as tile

BF16 = mybir.dt.bfloat16
F32 = mybir.dt.float32
FP8 = mybir.dt.float8e4
NPBF16 = ml_dtypes.bfloat16

N_EMBD = 1024
N_HEAD = 16
HS = 64
B = 2
T_FULL = 4096
QT = 512            # q-tile width
KTILE = 128         # k positions per chunk
N_CORES = 8

AluAdd = mybir.AluOpType.add
AluMult = mybir.AluOpType.mult


def build_nc(T=T_FULL):
    assert T % QT == 0
    NQT = T // QT          # q-tiles per unit (= per batch)
    NT = T // KTILE        # 128-wide k tiles per unit
    NQTR = NQT // 2        # quarters (j-pairs) per unit
    nc = bacc.Bacc()

    xT_d = nc.declare_dram_parameter("xT", [B * T // QT, 128, 8, QT], BF16, isOutput=False)
    wq_d = nc.declare_dram_parameter("Wq", [128, 8, 128], BF16, isOutput=False)
    wk_d = nc.declare_dram_parameter("Wk", [128, 8, 128], BF16, isOutput=False)
    wv_d = nc.declare_dram_parameter("Wv", [128, 8, 128], BF16, isOutput=False)
    bq_d = nc.declare_dram_parameter("bq", [128, 2], F32, isOutput=False)
    bk_d = nc.declare_dram_parameter("bk", [128, 2], F32, isOutput=False)
    bv_d = nc.declare_dram_parameter("bv_bc", [128, 128], F32, isOutput=False)
    cos_d = nc.declare_dram_parameter("cosT", [32, T], BF16, isOutput=False)
    sin_d = nc.declare_dram_parameter("sinT", [64, T], BF16, isOutput=False)
    tri_d = nc.declare_dram_parameter("tri", [128, 2, 128], BF16, isOutput=False)
    wp_d = nc.declare_dram_parameter("Wp", [128, 8, N_EMBD], BF16, isOutput=False)
    bp_d = nc.declare_dram_parameter("bp_bc", [128, N_EMBD], BF16, isOutput=False)
    # out rows: [unit, quarter, 128 tokens of this core's block, N_EMBD]
    out_d = nc.declare_dram_parameter("out", [B, NQTR, 128, N_EMBD], F32, isOutput=True)

    rec_d = nc.dram_tensor("rec_scratch", [B, NQT, 2, QT], F32)
    sync_in = nc.dram_tensor("sync_in", [8, 1, 16], BF16)
    sync_out = nc.dram_tensor("sync_out", [8, 1, 16], BF16)
    # per-quarter all-to-all staging: [slot, 2 heads x 64 dims, 128 tokens]
    a2a_in = [[nc.dram_tensor(f"a2a_in{u}{q}", [8, 128, 128], BF16) for q in range(NQTR)]
              for u in range(B)]
    a2a_out = [[nc.dram_tensor(f"a2a_out{u}{q}", [8, 128, 128], BF16) for q in range(NQTR)]
               for u in range(B)]

    with tile.TileContext(nc) as tc, \
         tc.tile_pool(name="const", bufs=1) as const, \
         tc.tile_pool(name="persist", bufs=1) as persist, \
         tc.tile_pool(name="xc", bufs=4) as xpool, \
         tc.tile_pool(name="stage", bufs=3) as stage, \
         tc.tile_pool(name="ppool", bufs=3) as ppool, \
         tc.tile_pool(name="epi", bufs=2) as epi, \
         tc.tile_pool(name="projp", bufs=2) as projp, \
         tc.tile_pool(name="qkv_ps", bufs=2, space="PSUM") as qkv_ps, \
         tc.tile_pool(name="strip_ps", bufs=2, space="PSUM") as strip_ps, \
         tc.tile_pool(name="av_ps", bufs=1, space="PSUM") as av_ps:

        # ---- constants.  Scalar queue: weights (in first-use order).
        # Vector queue: cos/sin loads + on-chip partition replication.
        # Sync queue: xc chunk DMAs.  GpSimd queue: collectives only. ----
        wq_sb = const.tile([128, 8, 128], BF16, tag="wq")
        wk_sb = const.tile([128, 8, 128], BF16, tag="wk")
        wv_sb = const.tile([128, 8, 128], BF16, tag="wv")
        bq_sb = const.tile([128, 2], F32, tag="bq")
        bk_sb = const.tile([128, 2], F32, tag="bk")
        bv_sb = const.tile([128, 128], F32, tag="bv")
        tri_sb = const.tile([128, 2, 128], BF16, tag="tri")
        nc.scalar.dma_start(out=wq_sb[:], in_=wq_d[:])
        nc.scalar.dma_start(out=bq_sb[:], in_=bq_d[:])
        nc.scalar.dma_start(out=wk_sb[:], in_=wk_d[:])
        nc.scalar.dma_start(out=bk_sb[:], in_=bk_d[:])
        nc.scalar.dma_start(out=wv_sb[:], in_=wv_d[:])
        nc.scalar.dma_start(out=bv_sb[:], in_=bv_d[:])
        nc.scalar.dma_start(out=tri_sb[:], in_=tri_d[:])

        cos_sb = const.tile([128, T], BF16, tag="cos")
        sin_sb = const.tile([128, T], BF16, tag="sin")
        nc.gpsimd.dma_start(out=cos_sb[0:32, :], in_=cos_d[:])
        nc.gpsimd.dma_start(out=sin_sb[0:64, :], in_=sin_d[:])
        nc.gpsimd.dma_start(out=cos_sb[32:64, :], in_=cos_sb[0:32, :])
        nc.gpsimd.dma_start(out=cos_sb[64:128, :], in_=cos_sb[0:64, :])
        nc.gpsimd.dma_start(out=sin_sb[64:128, :], in_=sin_sb[0:64, :])

        # proj weights load later (background piece) to keep boot HBM free
        wp_sb = const.tile([128, 8, N_EMBD], BF16, tag="wp")
        bp_sb = const.tile([128, N_EMBD], BF16, tag="bp")

        def load_wp():
            nc.scalar.dma_start(out=wp_sb[:], in_=wp_d[:])
            nc.scalar.dma_start(out=bp_sb[:], in_=bp_d[:])

        # ---- persistent per-unit tensors ----
        qT = [persist.tile([128, T], BF16, tag=f"qT{u}", name=f"qT{u}") for u in range(B)]
        kT = [persist.tile([128, T], BF16, tag=f"kT{u}", name=f"kT{u}") for u in range(B)]
        # v in fp8, chunk-paired for DoubleRow AV: [token, pair, ko, 2 heads x 80]
        # head h of k-chunk (2*pair+ko) lives at [64 dims + ones col + pad]
        vP = [persist.tile([128, NT // 2, 2, 160], FP8, tag=f"vP{u}", name=f"vP{u}")
              for u in range(B)]
        # exact bf16 v for chunks 0-3: the j=0 q-block averages few positions,
        # so fp8 v quantization would show through unaveraged there
        vP16 = [persist.tile([128, 4, 130], BF16, tag=f"vP16{u}", name=f"vP16{u}")
                for u in range(B)]
        # normalized y^T: [64 dims, head, T]
        yT2 = [persist.tile([64, 2, T], BF16, tag=f"yT{u}", name=f"yT{u}") for u in range(B)]
        for u in range(B):
            nc.vector.memset(vP[u][:, :, :, 64:65], 1.0)
            nc.vector.memset(vP[u][:, :, :, 144:145], 1.0)
            nc.vector.memset(vP16[u][:, :, 64:65], 1.0)
            nc.vector.memset(vP16[u][:, :, 129:130], 1.0)
        # tiny warmup all-to-all: absorbs boot-time core skew during the idle
        # head so the first real collective doesn't stall mid-attention
        nc.gpsimd.collective_compute(
            "AllToAll", mybir.AluOpType.bypass,
            replica_groups=[[0, 1, 2, 3, 4, 5, 6, 7]],
            ins=[sync_in[:]], outs=[sync_out[:]],
        )

        def qkv_chunk_pieces(u, ch):
            """Pieces (closures) computing qT/kT/vP for tokens [ch*512,(ch+1)*512) of unit u."""
            st = {}

            def p_xc():
                chg = u * (T // QT) + ch
                xc = xpool.tile([128, 8, QT], BF16, tag="xc", name="xc")
                nc.sync.dma_start(out=xc[:], in_=xT_d[chg])
                st["xc"] = xc

            def mk_qk(w_sb, b_sb, dstT):
                def p_qk():
                    csl = slice(ch * QT, (ch + 1) * QT)
                    xc = st["xc"]
                    pA = qkv_ps.tile([128, QT], F32, tag="qk", name="pA")
                    for ct in range(8):
                        nc.tensor.matmul(pA[:], w_sb[:, ct, :], xc[:, ct, :],
                                         start=(ct == 0), stop=(ct == 7))
                    qa = stage.tile([128, QT], BF16, tag="qa", name="qa", bufs=3)
                    nc.vector.tensor_scalar_add(qa[:], pA[:], b_sb[:, 0:1])
                    qr = stage.tile([128, QT], BF16, tag="qr", name="qr", bufs=3)
                    for (dp, sp) in ((0, 32), (32, 0), (64, 96), (96, 64)):
                        nc.sync.dma_start(out=qr[dp:dp + 32, :], in_=qa[sp:sp + 32, :])
                    m1 = stage.tile([128, QT], BF16, tag="m1", name="m1", bufs=2)
                    m2 = stage.tile([128, QT], BF16, tag="m2", name="m2", bufs=2)
                    nc.vector.tensor_mul(m1[:], qa[:], cos_sb[:, csl])
                    nc.vector.tensor_mul(m2[:], qr[:], sin_sb[:, csl])
                    nc.vector.tensor_add(dstT[:, csl], m1[:], m2[:])
                return p_qk

            def mk_v(t4):
                def p_v():
                    xc = st["xc"]
                    ttg = ch * 4 + t4
                    tsl = slice(t4 * 128, (t4 + 1) * 128)
                    pV = qkv_ps.tile([128, QT], F32, tag="qk", name="pV")
                    for ct in range(8):
                        nc.tensor.matmul(pV[:, 0:128], xc[:, ct, tsl], wv_sb[:, ct, :],
                                         start=(ct == 0), stop=(ct == 7))
                    pv2 = bass.AP(tensor=pV.tensor, offset=pV.offset,
                                  ap=[list(pV.ap[0]), [64, 2], [1, 64]])
                    dst = vP[u][:, ttg // 2, ttg % 2, 0:64]
                    dst2 = bass.AP(tensor=dst.tensor, offset=dst.offset,
                                   ap=[list(dst.ap[0]), [80, 2], [1, 64]])
                    bv2 = bass.AP(tensor=bv_sb.tensor, offset=bv_sb.offset,
                                  ap=[list(bv_sb.ap[0]), [64, 2], [1, 64]])
                    nc.vector.tensor_add(dst2, pv2, bv2)
                    if ttg < 4:
                        d16 = vP16[u][:, ttg, 0:129]
                        d16b = bass.AP(tensor=d16.tensor, offset=d16.offset,
                                       ap=[list(d16.ap[0]), [65, 2], [1, 64]])
                        nc.vector.tensor_add(d16b, pv2, bv2)
                return p_v

            return [p_xc, mk_qk(wq_sb, bq_sb, qT[u]), mk_qk(wk_sb, bk_sb, kT[u])] + \
                [mk_v(t4) for t4 in range(4)]

        def attn_block(u, j, drain):
            """Attention for q-tile j (512 q) of unit u against k tiles 0..4j+3.
            Calls drain() between chunks to interleave background PE work."""
            jsl = slice(j * QT, (j + 1) * QT)
            nchunks = 4 * (j + 1)
            npairs = nchunks // 2
            av_t = av_ps.tile([128, 2, QT], F32, tag="av", name="av_t")

            def av_emit(pend):
                # one DoubleRow matmul per head contracts both chunks of the pair
                P8, p, pqoff = pend
                w = QT - pqoff
                first, last = (p == 0), (p == npairs - 1)
                for h in range(2):
                    vb = vP[u][:, p, 0, 0:65]
                    lhsT = bass.AP(tensor=vb.tensor, offset=vb.offset + h * 80,
                                   ap=[list(vb.ap[0]), [160, 2], [1, 65]])
                    pb = P8[:, 0, h, 0:QT]
                    rhs = bass.AP(tensor=pb.tensor, offset=pb.offset + pqoff,
                                  ap=[list(pb.ap[0]), [2 * QT, 2], [1, w]])
                    nc.tensor.matmul(av_t[0:65, h, pqoff:QT], lhsT, rhs,
                                     start=first, stop=last,
                                     perf_mode=mybir.MatmulPerfMode.DoubleRow)

            def score_mms(c, qoff, w):
                ksl = slice(c * KTILE, (c + 1) * KTILE)
                strip = strip_ps.tile([128, 2, QT], F32, tag="strip", name="strip")
                for h in range(2):
                    hsl = slice(64 * h, 64 * (h + 1))
                    nc.tensor.matmul(strip[:, h, 0:w], kT[u][hsl, ksl],
                                     qT[u][hsl, j * QT + qoff:(j + 1) * QT],
                                     start=True, stop=True)
                return strip

            if j == 0:
                # exact bf16 path for the first q-block (4 chunks, all diagonal)
                def av_emit16(pend16):
                    P, c, qoff, w = pend16
                    first, last = (c == 0), (c == nchunks - 1)
                    for h in range(2):
                        nc.tensor.matmul(av_t[0:65, h, qoff:QT],
                                         vP16[u][:, c, 65 * h:65 * h + 65],
                                         P[:, h, 0:w], start=first, stop=last)

                pend16 = None
                for c in range(nchunks):
                    qoff = 128 * c if c > 0 else 0
                    w = QT - qoff
                    strip = score_mms(c, qoff, w)
                    P = ppool.tile([128, 2, QT], BF16, tag="P", name="P", bufs=3)
                    nc.scalar.activation(P[:, :, 0:w], strip[:, :, 0:w],
                                         mybir.ActivationFunctionType.Exp)
                    nc.vector.tensor_mul(P[:, :, 0:128], P[:, :, 0:128], tri_sb[:])
                    if pend16 is not None:
                        av_emit16(pend16)
                    pend16 = (P, c, qoff, w)
                av_emit16(pend16)
            else:
                pend = None
                P8 = None
                pair_qoff = 0
                for c in range(nchunks):
                    s = c - (nchunks - 4)      # diagonal sub-position 0..3, or <0
                    qoff = 128 * s if s > 0 else 0
                    w = QT - qoff
                    pk = c % 2
                    if pk == 0:
                        # fp8 P for the chunk pair: [token, ko(parity), head, q]
                        P8 = ppool.tile([128, 2, 2, QT], FP8, tag="P", name="P", bufs=3)
                        pair_qoff = qoff
                    strip = score_mms(c, qoff, w)
                    nc.scalar.activation(P8[:, pk, :, qoff:QT], strip[:, :, 0:w],
                                         mybir.ActivationFunctionType.Exp)
                    if s >= 0:  # leading 128 cols of a diagonal chunk: triangle mask
                        nc.vector.tensor_mul(P8[:, pk, :, qoff:qoff + 128],
                                             P8[:, pk, :, qoff:qoff + 128], tri_sb[:])
                    if pk == 1:
                        if s >= 1:  # odd diagonal chunk: zero its fully-masked gap
                            nc.vector.memset(P8[:, 1, :, pair_qoff:qoff], 0.0)
                        if pend is not None:
                            av_emit(pend)
                        pend = (P8, c // 2, pair_qoff)
                    if c % 3 == 2:
                        drain(1)
                av_emit(pend)
            # epilogue: evacuate denominator+y (custom-DVE ops need base-aligned
            # partitions on HW, so the den hop to partition 0 is required),
            # reciprocal, DRAM-bounce partition broadcast, normalize.
            den = epi.tile([1, 2, QT], F32, tag="den", name="den")
            nc.vector.tensor_copy(den[:], av_t[64:65, :, :])
            rc = epi.tile([1, 2, QT], F32, tag="rc", name="rc")
            nc.vector.reciprocal_approx_fast(rc[0:1, :, :], den[0:1, :, :])
            yc = epi.tile([64, 2, QT], BF16, tag="yc", name="yc")
            nc.vector.tensor_copy(yc[:], av_t[0:64, :, :])
            rb = epi.tile([64, 2, QT], F32, tag="rb", name="rb", bufs=1)
            nc.sync.dma_start(out=rec_d[u, j], in_=rc[0:1, :, :])
            dsrc = rec_d[u, j]
            bsrc = bass.AP(tensor=dsrc.tensor, offset=dsrc.offset,
                           ap=[[0, 64]] + list(dsrc.ap))
            nc.sync.dma_start(out=rb[:], in_=bsrc)
            nc.vector.tensor_mul(yT2[u][:, :, jsl], yc[:], rb[:])
            # stage into the quarter's all-to-all buffer:
            # a2a_in[u][q][s, 64h+r, t] = yT2[u][r, h, j*512 + (s-4*(j%2))*128 + t]
            q2, jh = j // 2, j % 2
            t_in = a2a_in[u][q2][:]
            for h in range(2):
                dst = bass.AP(tensor=t_in.tensor,
                              offset=t_in.offset + jh * 4 * 16384 + h * 8192,
                              ap=[[128, 64], [16384, 4], [1, 128]])
                qdma = nc.sync.dma_start if h == 0 else nc.scalar.dma_start
                qdma(out=dst, in_=yT2[u][:, h, jsl])

        def a2a_start(u, q):
            nc.gpsimd.collective_compute(
                "AllToAll", mybir.AluOpType.bypass,
                replica_groups=[[0, 1, 2, 3, 4, 5, 6, 7]],
                ins=[a2a_in[u][q][:]], outs=[a2a_out[u][q][:]],
            )

        def proj_mtile(u, q):
            """out rows for this core's 128-token block of quarter q of unit u."""
            ydm = projp.tile([128, 8, 128], BF16, tag="ydm", name="ydm")
            t_out = a2a_out[u][q][:]
            # ydm[d, s, t] = a2a_out[u][q][s, d, t]  (one DMA)
            src3 = bass.AP(tensor=t_out.tensor, offset=t_out.offset,
                           ap=[[128, 128], [16384, 8], [1, 128]])
            nc.scalar.dma_start(out=ydm[:], in_=src3)
            ob = projp.tile([128, N_EMBD], F32, tag="ob", name="ob", bufs=1)
            for nh2 in range(2):
                nsl = slice(nh2 * 512, (nh2 + 1) * 512)
                pp = qkv_ps.tile([128, QT], F32, tag="qk", name="pp")
                for ft in range(8):
                    nc.tensor.matmul(pp[:], ydm[:, ft, :], wp_sb[:, ft, nsl],
                                     start=(ft == 0), stop=(ft == 7))
                nc.vector.tensor_add(ob[:, nsl], pp[:], bp_sb[:, nsl])
            nc.sync.dma_start(out=out_d[u, q], in_=ob[:])

        # ---- schedule: attention paces ScalarE; qkv/proj pieces fill PE gaps ----
        bg = []          # list of (key, piece_fn); key=(u, ch) for qkv, None otherwise
        bgi = [0]
        qkv_done = {}    # u -> highest chunk fully emitted

        def drain(n):
            for _ in range(n):
                if bgi[0] >= len(bg):
                    return
                key, fn = bg[bgi[0]]
                bgi[0] += 1
                fn()
                if key is not None:
                    qkv_done[key[0]] = key[1]

        def add_chunk(u, ch):
            ps = qkv_chunk_pieces(u, ch)
            # key only on the LAST piece: chunk counts as emitted when all pieces ran
            bg.extend((None, p) for p in ps[:-1])
            bg.append(((u, ch), ps[-1]))

        ps0 = qkv_chunk_pieces(0, 0)
        for p in ps0:
            p()
        qkv_done[0] = 0
        add_chunk(0, 1)
        bg.append((None, load_wp))
        for ch in range(2, NQT):
            add_chunk(0, ch)
        for ch in range(NQT):
            add_chunk(1, ch)

        n_coll = 2 * NQTR
        emitted = set()
        for u in range(B):
            for j in range(NQT):
                while qkv_done.get(u, -1) < j:
                    drain(1)
                attn_block(u, j, drain)
                if j % 2 == 1:
                    q = j // 2
                    k = NQTR * u + q
                    if k < n_coll - 1:  # the final collective is issued below
                        a2a_start(u, q)
                    if k >= 2:
                        pu, pq = divmod(k - 2, NQTR)
                        emitted.add(k - 2)
                        bg.append((None, lambda pu=pu, pq=pq: proj_mtile(pu, pq)))
        while bgi[0] < len(bg):
            drain(1)
        # emit remaining earlier-quarter proj BEFORE the final collective so
        # their a2a_out reads don't serialize behind it (coarse DRAM aliasing);
        # the final quarter's proj fills the final collective's latency.
        for k in range(n_coll - 1):
            if k not in emitted:
                pu, pq = divmod(k, NQTR)
                proj_mtile(pu, pq)
        a2a_start(B - 1, NQTR - 1)
        proj_mtile(B - 1, NQTR - 1)

    nc.compile()
    return nc


def make_inputs(x, W_attn, b_attn, W_proj, b_proj, T):
    """Build the 8 per-core input maps from full inputs."""
    scale = 1.0 / np.sqrt(HS)
    inv_freq = 1.0 / (10000.0 ** (np.arange(0, HS, 2, dtype=np.float64) / HS))  # [32]
    t = np.arange(T, dtype=np.float64)
    freqs = np.outer(t, inv_freq)  # [T, 32]
    cos32 = np.cos(freqs).T.astype(np.float32)               # [32, T]
    sin32 = np.sin(freqs).T.astype(np.float32)               # [32, T]
    sin64 = np.concatenate([-sin32, sin32], axis=0)          # [64, T]

    # triangle mask for the leading 128 cols of diagonal chunks: 1 iff p <= f
    p = np.arange(128)[:, None]
    f = np.arange(128)[None, :]
    tri = (p <= f).astype(np.float32)
    tri2 = np.stack([tri, tri], axis=1)  # [128, 2, 128]

    C = N_EMBD
    # chunk-major x: xh[ch, p, a, t] = x[ch*QT + t, a*128 + p]
    xh = np.ascontiguousarray(
        x.reshape(B * T // QT, QT, 8, 128).transpose(0, 3, 2, 1)).astype(NPBF16)
    # rot permutation of head dims: d -> d+32 (first half) / d-32 (second half)
    d = np.arange(128)
    perm = np.where((d % 64) < 32, d + 32, d - 32)
    bp_bc = np.broadcast_to(b_proj[None, :], (128, N_EMBD)).astype(np.float32).copy()
    in_maps = []
    for c in range(N_CORES):
        hsl = slice(128 * c, 128 * (c + 1))  # dims of heads {2c, 2c+1}
        Wq = W_attn[:, 0 * C:1 * C][:, hsl] * scale
        Wk = W_attn[:, 1 * C:2 * C][:, hsl]
        Wv = W_attn[:, 2 * C:3 * C][:, hsl]
        bq = (b_attn[0 * C:1 * C][hsl] * scale).astype(np.float32)
        bk = b_attn[1 * C:2 * C][hsl].astype(np.float32)
        bv = b_attn[2 * C:3 * C][hsl]
        in_maps.append({
            "xT": xh,
            "Wq": np.ascontiguousarray(Wq.reshape(8, 128, 128).transpose(1, 0, 2)).astype(NPBF16),
            "Wk": np.ascontiguousarray(Wk.reshape(8, 128, 128).transpose(1, 0, 2)).astype(NPBF16),
            "Wv": np.ascontiguousarray(Wv.reshape(8, 128, 128).transpose(1, 0, 2)).astype(NPBF16),
            "bq": np.stack([bq, bq[perm]], axis=1).copy(),
            "bk": np.stack([bk, bk[perm]], axis=1).copy(),
            "bv_bc": np.broadcast_to(bv[None, :], (128, 128)).astype(np.float32).copy(),
            "cosT": cos32.astype(NPBF16),
            "sinT": sin64.astype(NPBF16),
            "tri": tri2.astype(NPBF16),
            "Wp": np.ascontiguousarray(W_proj.reshape(8, 128, N_EMBD).transpose(1, 0, 2)).astype(NPBF16),
            "bp_bc": bp_bc.astype(NPBF16),
        })
    return in_maps


def assemble(results, T):
    NQTR = T // QT // 2
    out = np.empty((B, T, N_EMBD), dtype=np.float32)
    for c in range(N_CORES):
        blk = results[c]["out"]  # [B, NQTR, 128, N_EMBD]
        for u in range(B):
            for q in range(NQTR):
                out[u, q * 1024 + c * 128:q * 1024 + (c + 1) * 128, :] = blk[u, q]
    return out


_NC_CACHE = {}


def kernel(x, W_attn, b_attn, W_proj, b_proj):
    from concourse.bass_utils import run_bass_kernel_spmd
    x = np.asarray(x, dtype=np.float32)
    W_attn = np.asarray(W_attn, dtype=np.float32)
    b_attn = np.asarray(b_attn, dtype=np.float32)
    W_proj = np.asarray(W_proj, dtype=np.float32)
    b_proj = np.asarray(b_proj, dtype=np.float32)
    T = x.shape[1]
    if T not in _NC_CACHE:
        _NC_CACHE[T] = build_nc(T)
    nc = _NC_CACHE[T]
    in_maps = make_inputs(x, W_attn, b_attn, W_proj, b_proj, T)
    res = run_bass_kernel_spmd(nc, in_maps, core_ids=list(range(N_CORES)))
    return assemble(res.results, T)


# revision 41
# speedup vs baseline: 1.0576x; 1.0063x over previous
"""Trainium2 Bass kernel for CausalSelfAttention (B=2, T=4096, C=1024, 16 heads, RoPE).

Sharding: tensor-parallel across heads. Core c handles heads {2c, 2c+1} for
both batches; the two batches are processed as two "units".

Per core (v3 — tightened head/tail schedule):
  - QKV emitted as pieces drained into the attention stream (PE never idles):
    qT/kT computed transposed ([dims, T]), v natural. RoPE via partition-
    rotated copy (sign folded into the bf16 sin table). cos/sin tables are
    loaded at 32/64 distinct rows and partition-replicated on-chip.
  - Attention paces ScalarE: per 128-k chunk, a row-tiled score matmul pair
    (both heads on PE row tiles) into a [128, 2, 512] PSUM strip, exp on
    ScalarE, triangle mask on diagonal chunks, K=128 AV matmuls accumulating
    y + softmax denominator (65th ones column) into a [128, 2, 512] PSUM
    accumulator.
  - Epilogue: reciprocal straight off the PSUM denominator row, SBUF->SBUF
    partition broadcast, one fused normalize multiply into yT2, two
    a2a-staging DMAs (split across queues).
  - Eight per-quarter 8-way AllToAlls (one per 1024 tokens of a unit,
    y^T head-sharded -> token-sharded); proj m-tiles run in the background
    stream two collectives behind, so only the last quarter's collective and
    one proj tile are exposed at the tail.

PSUM budget (8 banks): 2x2 score strips + 2 AV accumulator + 2 qkv/proj ring.
kernel() takes the full unsharded inputs and returns the full output.
"""

import numpy as np
import ml_dtypes

import concourse.bass as bass
import concourse.bacc as bacc
import concourse.mybir as mybir
import concourse.tile as tile

BF16 = mybir.dt.bfloat16
F32 = mybir.dt.float32
FP8 = mybir.dt.float8e4
NPBF16 = ml_dtypes.bfloat16
NPFP8 = ml_dtypes.float8_e4m3
QK_SCALE = 16.0      # q/k weights are scaled up into fp8's normal range
EXP_SCALE = 1.0 / (QK_SCALE * QK_SCALE * 8.0)   # folds 1/sqrt(hs) too

N_EMBD = 1024
N_HEAD = 16
HS = 64
B = 2
T_FULL = 4096
QT = 512            # q-tile width
KTILE = 128         # k positions per chunk
N_CORES = 8

AluAdd = mybir.AluOpType.add
AluMult = mybir.AluOpType.mult


def build_nc(T=T_FULL):
    assert T % QT == 0
    NQT = T // QT          # q-tiles per unit (= per batch)
    NT = T // KTILE        # 128-wide k tiles per unit
    NQTR = NQT // 2        # quarters (j-pairs) per unit
    nc = bacc.Bacc()

    xT_d = nc.declare_dram_parameter("xT", [B * T // QT, 128, 8, QT], BF16, isOutput=False)
    xT8_d = nc.declare_dram_parameter("xT8", [B * T // QT, 128, 4, 2, QT], FP8, isOutput=False)
    wq_d = nc.declare_dram_parameter("Wq", [128, 4, 2, 128], FP8, isOutput=False)
    wk_d = nc.declare_dram_parameter("Wk", [128, 4, 2, 128], FP8, isOutput=False)
    wv_d = nc.declare_dram_parameter("Wv", [128, 8, 128], BF16, isOutput=False)
    wq16_d = nc.declare_dram_parameter("Wq16", [128, 8, 128], BF16, isOutput=False)
    wk16_d = nc.declare_dram_parameter("Wk16", [128, 8, 128], BF16, isOutput=False)
    bq_d = nc.declare_dram_parameter("bq", [128, 2], F32, isOutput=False)
    bk_d = nc.declare_dram_parameter("bk", [128, 2], F32, isOutput=False)
    bv_d = nc.declare_dram_parameter("bv_bc", [128, 128], F32, isOutput=False)
    cos_d = nc.declare_dram_parameter("cosT", [32, T], BF16, isOutput=False)
    sin_d = nc.declare_dram_parameter("sinT", [64, T], BF16, isOutput=False)
    tri_d = nc.declare_dram_parameter("tri", [128, 2, 128], BF16, isOutput=False)
    wp_d = nc.declare_dram_parameter("Wp", [128, 8, N_EMBD], BF16, isOutput=False)
    bp_d = nc.declare_dram_parameter("bp_bc", [128, N_EMBD], BF16, isOutput=False)
    # out rows: [unit, quarter, 128 tokens of this core's block, N_EMBD]
    out_d = nc.declare_dram_parameter("out", [B, NQTR, 128, N_EMBD], F32, isOutput=True)

    sync_in = nc.dram_tensor("sync_in", [8, 1, 16], BF16)
    sync_out = nc.dram_tensor("sync_out", [8, 1, 16], BF16)
    rec_d = nc.dram_tensor("rec_scratch", [B, 8, 2, QT], F32)
    # per-quarter all-to-all staging: [slot, 2 heads x 64 dims, 128 tokens]
    a2a_in = [[nc.dram_tensor(f"a2a_in{u}{q}", [8, 128, 128], BF16) for q in range(NQTR)]
              for u in range(B)]
    a2a_out = [[nc.dram_tensor(f"a2a_out{u}{q}", [8, 128, 128], BF16) for q in range(NQTR)]
               for u in range(B)]

    with tile.TileContext(nc) as tc, \
         tc.tile_pool(name="const", bufs=1) as const, \
         tc.tile_pool(name="persist", bufs=1) as persist, \
         tc.tile_pool(name="xc", bufs=4) as xpool, \
         tc.tile_pool(name="stage", bufs=3) as stage, \
         tc.tile_pool(name="ppool", bufs=3) as ppool, \
         tc.tile_pool(name="epi", bufs=2) as epi, \
         tc.tile_pool(name="projp", bufs=2) as projp, \
         tc.tile_pool(name="qkv_ps", bufs=2, space="PSUM") as qkv_ps, \
         tc.tile_pool(name="strip_ps", bufs=2, space="PSUM") as strip_ps, \
         tc.tile_pool(name="av_ps", bufs=1, space="PSUM") as av_ps:

        # ---- constants.  Scalar queue: weights (in first-use order).
        # Vector queue: cos/sin loads + on-chip partition replication.
        # Sync queue: xc chunk DMAs.  GpSimd queue: collectives only. ----
        wq_sb = const.tile([128, 4, 2, 128], FP8, tag="wq")
        wk_sb = const.tile([128, 4, 2, 128], FP8, tag="wk")
        wv_sb = const.tile([128, 8, 128], BF16, tag="wv")
        bq_sb = const.tile([128, 2], F32, tag="bq")
        bk_sb = const.tile([128, 2], F32, tag="bk")
        bv_sb = const.tile([128, 128], F32, tag="bv")
        tri_sb = const.tile([128, 2, 128], BF16, tag="tri")
        # exact bf16 q/k weights for chunk 0 (feeds the sensitive j=0 block)
        wq16_sb = const.tile([128, 8, 128], BF16, tag="wq16")
        wk16_sb = const.tile([128, 8, 128], BF16, tag="wk16")
        nc.scalar.dma_start(out=wq16_sb[:], in_=wq16_d[:])
        nc.scalar.dma_start(out=bq_sb[:], in_=bq_d[:])
        nc.scalar.dma_start(out=wk16_sb[:], in_=wk16_d[:])
        nc.scalar.dma_start(out=bk_sb[:], in_=bk_d[:])
        nc.scalar.dma_start(out=wv_sb[:], in_=wv_d[:])
        nc.scalar.dma_start(out=bv_sb[:], in_=bv_d[:])
        nc.scalar.dma_start(out=tri_sb[:], in_=tri_d[:])
        nc.scalar.dma_start(out=wq_sb[:], in_=wq_d[:])
        nc.scalar.dma_start(out=wk_sb[:], in_=wk_d[:])

        cos_sb = const.tile([128, T], BF16, tag="cos")
        sin_sb = const.tile([128, T], BF16, tag="sin")
        nc.gpsimd.dma_start(out=cos_sb[0:32, :], in_=cos_d[:])
        nc.gpsimd.dma_start(out=sin_sb[0:64, :], in_=sin_d[:])
        nc.gpsimd.dma_start(out=cos_sb[32:64, :], in_=cos_sb[0:32, :])
        nc.gpsimd.dma_start(out=cos_sb[64:128, :], in_=cos_sb[0:64, :])
        nc.gpsimd.dma_start(out=sin_sb[64:128, :], in_=sin_sb[0:64, :])

        # proj weights load later (background piece) to keep boot HBM free
        wp_sb = const.tile([128, 8, N_EMBD], BF16, tag="wp")
        bp_sb = const.tile([128, N_EMBD], BF16, tag="bp")

        def load_wp():
            nc.scalar.dma_start(out=wp_sb[:], in_=wp_d[:])
            nc.scalar.dma_start(out=bp_sb[:], in_=bp_d[:])

        # ---- persistent per-unit tensors ----
        qT = [persist.tile([128, T], BF16, tag=f"qT{u}", name=f"qT{u}") for u in range(B)]
        kT = [persist.tile([128, T], BF16, tag=f"kT{u}", name=f"kT{u}") for u in range(B)]
        # v in fp8, chunk-paired for DoubleRow AV: [token, pair, ko, 2 heads x 80]
        # head h of k-chunk (2*pair+ko) lives at [64 dims + ones col + pad]
        vP = [persist.tile([128, NT // 2, 2, 160], FP8, tag=f"vP{u}", name=f"vP{u}")
              for u in range(B)]
        # exact bf16 v for chunks 0-3: the j=0 q-block averages few positions,
        # so fp8 v quantization would show through unaveraged there
        vP16 = [persist.tile([128, 4, 130], BF16, tag=f"vP16{u}", name=f"vP16{u}")
                for u in range(B)]
        for u in range(B):
            nc.vector.memset(vP[u][:, :, :, 64:65], 1.0)
            nc.vector.memset(vP[u][:, :, :, 144:145], 1.0)
            nc.vector.memset(vP16[u][:, :, 64:65], 1.0)
            nc.vector.memset(vP16[u][:, :, 129:130], 1.0)
        # tiny warmup all-to-all: absorbs boot-time core skew during the idle
        # head so the first real collective doesn't stall mid-attention
        nc.gpsimd.collective_compute(
            "AllToAll", mybir.AluOpType.bypass,
            replica_groups=[[0, 1, 2, 3, 4, 5, 6, 7]],
            ins=[sync_in[:]], outs=[sync_out[:]],
        )

        def qkv_chunk_pieces(u, ch):
            """Pieces (closures) computing qT/kT/vP for tokens [ch*512,(ch+1)*512) of unit u."""
            st = {}

            def p_xc():
                chg = u * (T // QT) + ch
                xc = xpool.tile([128, 8, QT], BF16, tag="xc", name="xc")
                nc.sync.dma_start(out=xc[:], in_=xT_d[chg])
                st["xc"] = xc
                if ch > 0:
                    xc8 = xpool.tile([128, 4, 2, QT], FP8, tag="xc8", name="xc8")
                    nc.sync.dma_start(out=xc8[:], in_=xT8_d[chg])
                    st["xc8"] = xc8

            def mk_qk(w_sb, w16_sb, b_sb, dstT):
                def p_qk():
                    csl = slice(ch * QT, (ch + 1) * QT)
                    pA = qkv_ps.tile([128, QT], F32, tag="qk", name="pA")
                    if ch == 0:
                        xc = st["xc"]
                        for ct in range(8):
                            nc.tensor.matmul(pA[:], w16_sb[:, ct, :], xc[:, ct, :],
                                             start=(ct == 0), stop=(ct == 7))
                    else:
                        xc8 = st["xc8"]
                        for g in range(4):
                            nc.tensor.matmul(pA[:], w_sb[:, g, :, :], xc8[:, g, :, :],
                                             start=(g == 0), stop=(g == 3),
                                             perf_mode=mybir.MatmulPerfMode.DoubleRow)
                    qa = stage.tile([128, QT], BF16, tag="qa", name="qa", bufs=3)
                    nc.vector.tensor_scalar_add(qa[:], pA[:], b_sb[:, 0:1])
                    qr = stage.tile([128, QT], BF16, tag="qr", name="qr", bufs=3)
                    for (dp, sp) in ((0, 32), (32, 0), (64, 96), (96, 64)):
                        nc.sync.dma_start(out=qr[dp:dp + 32, :], in_=qa[sp:sp + 32, :])
                    m1 = stage.tile([128, QT], BF16, tag="m1", name="m1", bufs=2)
                    m2 = stage.tile([128, QT], BF16, tag="m2", name="m2", bufs=2)
                    nc.vector.tensor_mul(m1[:], qa[:], cos_sb[:, csl])
                    nc.vector.tensor_mul(m2[:], qr[:], sin_sb[:, csl])
                    nc.vector.tensor_add(dstT[:, csl], m1[:], m2[:])
                return p_qk

            def mk_v(t4):
                def p_v():
                    xc = st["xc"]
                    ttg = ch * 4 + t4
                    tsl = slice(t4 * 128, (t4 + 1) * 128)
                    pV = qkv_ps.tile([128, QT], F32, tag="qk", name="pV")
                    for ct in range(8):
                        nc.tensor.matmul(pV[:, 0:128], xc[:, ct, tsl], wv_sb[:, ct, :],
                                         start=(ct == 0), stop=(ct == 7))
                    pv2 = bass.AP(tensor=pV.tensor, offset=pV.offset,
                                  ap=[list(pV.ap[0]), [64, 2], [1, 64]])
                    dst = vP[u][:, ttg // 2, ttg % 2, 0:64]
                    dst2 = bass.AP(tensor=dst.tensor, offset=dst.offset,
                                   ap=[list(dst.ap[0]), [80, 2], [1, 64]])
                    bv2 = bass.AP(tensor=bv_sb.tensor, offset=bv_sb.offset,
                                  ap=[list(bv_sb.ap[0]), [64, 2], [1, 64]])
                    nc.vector.tensor_add(dst2, pv2, bv2)
                    if ttg < 4:
                        d16 = vP16[u][:, ttg, 0:129]
                        d16b = bass.AP(tensor=d16.tensor, offset=d16.offset,
                                       ap=[list(d16.ap[0]), [65, 2], [1, 64]])
                        nc.vector.tensor_add(d16b, pv2, bv2)
                return p_v

            return [p_xc, mk_qk(wq_sb, wq16_sb, bq_sb, qT[u]),
                    mk_qk(wk_sb, wk16_sb, bk_sb, kT[u])] + \
                [mk_v(t4) for t4 in range(4)]

        def attn_block(u, j, drain):
            """Attention for q-tile j (512 q) of unit u against k tiles 0..4j+3.
            Calls drain() between chunks to interleave background PE work."""
            jsl = slice(j * QT, (j + 1) * QT)
            nchunks = 4 * (j + 1)
            npairs = nchunks // 2
            av_t = av_ps.tile([128, 2, QT], F32, tag="av", name="av_t")

            def av_emit(pend):
                # one DoubleRow matmul per head contracts both chunks of the pair
                P8, p, pqoff = pend
                w = QT - pqoff
                first, last = (p == 0), (p == npairs - 1)
                for h in range(2):
                    vb = vP[u][:, p, 0, 0:65]
                    lhsT = bass.AP(tensor=vb.tensor, offset=vb.offset + h * 80,
                                   ap=[list(vb.ap[0]), [160, 2], [1, 65]])
                    pb = P8[:, 0, h, 0:QT]
                    rhs = bass.AP(tensor=pb.tensor, offset=pb.offset + pqoff,
                                  ap=[list(pb.ap[0]), [2 * QT, 2], [1, w]])
                    nc.tensor.matmul(av_t[0:65, h, pqoff:QT], lhsT, rhs,
                                     start=first, stop=last,
                                     perf_mode=mybir.MatmulPerfMode.DoubleRow)

            def score_mms(c, qoff, w):
                ksl = slice(c * KTILE, (c + 1) * KTILE)
                strip = strip_ps.tile([128, 2, QT], F32, tag="strip", name="strip")
                for h in range(2):
                    hsl = slice(64 * h, 64 * (h + 1))
                    nc.tensor.matmul(strip[:, h, 0:w], kT[u][hsl, ksl],
                                     qT[u][hsl, j * QT + qoff:(j + 1) * QT],
                                     start=True, stop=True)
                return strip

            if j == 0:
                # exact bf16 path for the first q-block (4 chunks, all diagonal)
                def av_emit16(pend16):
                    P, c, qoff, w = pend16
                    first, last = (c == 0), (c == nchunks - 1)
                    for h in range(2):
                        nc.tensor.matmul(av_t[0:65, h, qoff:QT],
                                         vP16[u][:, c, 65 * h:65 * h + 65],
                                         P[:, h, 0:w], start=first, stop=last)

                pend16 = None
                for c in range(nchunks):
                    qoff = 128 * c if c > 0 else 0
                    w = QT - qoff
                    strip = score_mms(c, qoff, w)
                    P = ppool.tile([128, 2, QT], BF16, tag="P", name="P", bufs=3)
                    nc.scalar.activation(P[:, :, 0:w], strip[:, :, 0:w],
                                         mybir.ActivationFunctionType.Exp,
                                         scale=EXP_SCALE)
                    nc.vector.tensor_mul(P[:, :, 0:128], P[:, :, 0:128], tri_sb[:])
                    if pend16 is not None:
                        av_emit16(pend16)
                    pend16 = (P, c, qoff, w)
                av_emit16(pend16)
            else:
                pend = None
                P8 = None
                pair_qoff = 0
                for c in range(nchunks):
                    s = c - (nchunks - 4)      # diagonal sub-position 0..3, or <0
                    qoff = 128 * s if s > 0 else 0
                    w = QT - qoff
                    pk = c % 2
                    if pk == 0:
                        # fp8 P for the chunk pair: [token, ko(parity), head, q]
                        P8 = ppool.tile([128, 2, 2, QT], FP8, tag="P", name="P", bufs=3)
                        pair_qoff = qoff
                    strip = score_mms(c, qoff, w)
                    nc.scalar.activation(P8[:, pk, :, qoff:QT], strip[:, :, 0:w],
                                         mybir.ActivationFunctionType.Exp,
                                         scale=EXP_SCALE)
                    if s >= 0:  # leading 128 cols of a diagonal chunk: triangle mask
                        nc.vector.tensor_mul(P8[:, pk, :, qoff:qoff + 128],
                                             P8[:, pk, :, qoff:qoff + 128], tri_sb[:])
                    if pk == 1:
                        if s >= 1:  # odd diagonal chunk: zero its fully-masked gap
                            nc.vector.memset(P8[:, 1, :, pair_qoff:qoff], 0.0)
                        if pend is not None:
                            av_emit(pend)
                        pend = (P8, c // 2, pair_qoff)
                    if c % 3 == 2:
                        drain(1)
                av_emit(pend)
            # epilogue: evacuate denominator+y (frees the PSUM accumulator),
            # reciprocal, DRAM-bounce partition broadcast, normalize, stage.
            den = epi.tile([1, 2, QT], F32, tag="den", name="den")
            nc.vector.tensor_copy(den[:], av_t[64:65, :, :])
            yc = epi.tile([64, 2, QT], BF16, tag="yc", name="yc")
            nc.scalar.activation(yc[:], av_t[0:64, :, :],
                                 mybir.ActivationFunctionType.Copy)
            rc = epi.tile([1, 2, QT], F32, tag="rc", name="rc")
            nc.vector.reciprocal_approx_fast(rc[0:1, :, :], den[0:1, :, :])
            rb = epi.tile([64, 2, QT], F32, tag="rb", name="rb", bufs=1)
            nc.sync.dma_start(out=rec_d[u, j], in_=rc[0:1, :, :])
            dsrc = rec_d[u, j]
            bsrc = bass.AP(tensor=dsrc.tensor, offset=dsrc.offset,
                           ap=[[0, 64]] + list(dsrc.ap))
            nc.sync.dma_start(out=rb[:], in_=bsrc)
            yn = epi.tile([64, 2, QT], BF16, tag="yn", name="yn")
            nc.vector.tensor_mul(yn[:], yc[:], rb[:])
            # a2a_in[u][q][s, 64h+r, t] = yn[r, h, (s-4*(j%2))*128 + t]
            q2, jh = j // 2, j % 2
            t_in = a2a_in[u][q2][:]
            srow = 128 * 128
            for h in range(2):
                dst = bass.AP(tensor=t_in.tensor,
                              offset=t_in.offset + jh * 4 * srow + h * 64 * 128,
                              ap=[[128, 64], [srow, 4], [1, 128]])
                qdma = nc.sync.dma_start if h == 0 else nc.scalar.dma_start
                qdma(out=dst, in_=yn[:, h, :])

        def a2a_start(u, q):
            nc.gpsimd.collective_compute(
                "AllToAll", mybir.AluOpType.bypass,
                replica_groups=[[0, 1, 2, 3, 4, 5, 6, 7]],
                ins=[a2a_in[u][q][:]], outs=[a2a_out[u][q][:]],
            )

        def proj_mtile(u, q):
            """out rows for this core's 128-token block of quarter q of unit u."""
            t_out = a2a_out[u][q][:]
            srow = 128 * 128
            ydm = projp.tile([128, 8, 128], BF16, tag="ydm", name="ydm")
            # ydm[d, s, t] = a2a_out[u][q][s, d, t]  (one DMA)
            src3 = bass.AP(tensor=t_out.tensor, offset=t_out.offset,
                           ap=[[128, 128], [srow, 8], [1, 128]])
            nc.gpsimd.dma_start(out=ydm[:], in_=src3)
            ob = projp.tile([128, N_EMBD], F32, tag="ob", name="ob", bufs=1)
            for nh2 in range(2):
                nsl = slice(nh2 * N_EMBD // 2, (nh2 + 1) * N_EMBD // 2)
                pp = qkv_ps.tile([128, QT], F32, tag="qk", name="pp")
                for ft in range(8):
                    nc.tensor.matmul(pp[:], ydm[:, ft, :], wp_sb[:, ft, nsl],
                                     start=(ft == 0), stop=(ft == 7))
                nc.vector.tensor_add(ob[:, nsl], pp[:], bp_sb[:, nsl])
            nc.sync.dma_start(out=out_d[u, q], in_=ob[:])

        # ---- schedule: attention paces ScalarE; qkv/proj pieces fill PE gaps ----
        bg = []          # list of (key, piece_fn); key=(u, ch) for qkv, None otherwise
        bgi = [0]
        qkv_done = {}    # u -> highest chunk fully emitted

        def drain(n):
            for _ in range(n):
                if bgi[0] >= len(bg):
                    return
                key, fn = bg[bgi[0]]
                bgi[0] += 1
                fn()
                if key is not None:
                    qkv_done[key[0]] = key[1]

        def add_chunk(u, ch):
            ps = qkv_chunk_pieces(u, ch)
            # key only on the LAST piece: chunk counts as emitted when all pieces ran
            bg.extend((None, p) for p in ps[:-1])
            bg.append(((u, ch), ps[-1]))

        ps0 = qkv_chunk_pieces(0, 0)
        for p in ps0:
            p()
        qkv_done[0] = 0
        add_chunk(0, 1)
        bg.append((None, load_wp))
        for ch in range(2, NQT):
            add_chunk(0, ch)
        for ch in range(NQT):
            add_chunk(1, ch)

        n_coll = 2 * NQTR
        emitted = set()
        for u in range(B):
            for j in range(NQT):
                while qkv_done.get(u, -1) < j:
                    drain(1)
                attn_block(u, j, drain)
                if j % 2 == 1:
                    q = j // 2
                    k = NQTR * u + q
                    if k < n_coll - 1:  # the final collective is issued below
                        a2a_start(u, q)
                    if k >= 2:
                        pu, pq = divmod(k - 2, NQTR)
                        emitted.add(k - 2)
                        bg.append((None, lambda pu=pu, pq=pq: proj_mtile(pu, pq)))
        while bgi[0] < len(bg):
            drain(1)
        # emit remaining earlier-quarter proj BEFORE the final collective so
        # their a2a_out reads don't serialize behind it (coarse DRAM aliasing);
        # the final quarter's proj fills the final collective's latency.
        for k in range(n_coll - 1):
            if k not in emitted:
                pu, pq = divmod(k, NQTR)
                proj_mtile(pu, pq)
        a2a_start(B - 1, NQTR - 1)
        proj_mtile(B - 1, NQTR - 1)

    nc.compile()
    return nc


def make_inputs(x, W_attn, b_attn, W_proj, b_proj, T):
    """Build the 8 per-core input maps from full inputs."""
    inv_freq = 1.0 / (10000.0 ** (np.arange(0, HS, 2, dtype=np.float64) / HS))  # [32]
    t = np.arange(T, dtype=np.float64)
    freqs = np.outer(t, inv_freq)  # [T, 32]
    cos32 = np.cos(freqs).T.astype(np.float32)               # [32, T]
    sin32 = np.sin(freqs).T.astype(np.float32)               # [32, T]
    sin64 = np.concatenate([-sin32, sin32], axis=0)          # [64, T]

    # triangle mask for the leading 128 cols of diagonal chunks: 1 iff p <= f
    p = np.arange(128)[:, None]
    f = np.arange(128)[None, :]
    tri = (p <= f).astype(np.float32)
    tri2 = np.stack([tri, tri], axis=1)  # [128, 2, 128]

    C = N_EMBD
    # chunk-major x: xh[ch, p, a, t] = x[ch*QT + t, a*128 + p]
    xh = np.ascontiguousarray(
        x.reshape(B * T // QT, QT, 8, 128).transpose(0, 3, 2, 1)).astype(NPBF16)
    # fp8 copy in DoubleRow layout: [ch, p, g, ko, t], in-dim = (2g+ko)*128+p
    xh8 = np.ascontiguousarray(
        x.reshape(B * T // QT, QT, 4, 2, 128).transpose(0, 4, 2, 3, 1)).astype(NPFP8)

    def dr_w(W):
        # [1024 in, n out] -> [p, g, ko, n] with in = (2g+ko)*128+p
        n = W.shape[1]
        return np.ascontiguousarray(
            W.reshape(4, 2, 128, n).transpose(2, 0, 1, 3)).astype(NPFP8)

    # rot permutation of head dims: d -> d+32 (first half) / d-32 (second half)
    d = np.arange(128)
    perm = np.where((d % 64) < 32, d + 32, d - 32)
    bp_bc = np.broadcast_to(b_proj[None, :], (128, N_EMBD)).astype(np.float32).copy()
    in_maps = []
    for c in range(N_CORES):
        hsl = slice(128 * c, 128 * (c + 1))  # dims of heads {2c, 2c+1}
        Wq = W_attn[:, 0 * C:1 * C][:, hsl] * QK_SCALE
        Wk = W_attn[:, 1 * C:2 * C][:, hsl] * QK_SCALE
        Wv = W_attn[:, 2 * C:3 * C][:, hsl]
        bq = (b_attn[0 * C:1 * C][hsl] * QK_SCALE).astype(np.float32)
        bk = (b_attn[1 * C:2 * C][hsl] * QK_SCALE).astype(np.float32)
        bv = b_attn[2 * C:3 * C][hsl]
        in_maps.append({
            "xT": xh,
            "xT8": xh8,
            "Wq": dr_w(Wq),
            "Wk": dr_w(Wk),
            "Wq16": np.ascontiguousarray(Wq.reshape(8, 128, 128).transpose(1, 0, 2)).astype(NPBF16),
            "Wk16": np.ascontiguousarray(Wk.reshape(8, 128, 128).transpose(1, 0, 2)).astype(NPBF16),
            "Wv": np.ascontiguousarray(Wv.reshape(8, 128, 128).transpose(1, 0, 2)).astype(NPBF16),
            "bq": np.stack([bq, bq[perm]], axis=1).copy(),
            "bk": np.stack([bk, bk[perm]], axis=1).copy(),
            "bv_bc": np.broadcast_to(bv[None, :], (128, 128)).astype(np.float32).copy(),
            "cosT": cos32.astype(NPBF16),
            "sinT": sin64.astype(NPBF16),
            "tri": tri2.astype(NPBF16),
            "Wp": np.ascontiguousarray(W_proj.reshape(8, 128, N_EMBD).transpose(1, 0, 2)).astype(NPBF16),
            "bp_bc": bp_bc.astype(NPBF16),
        })
    return in_maps


def assemble(results, T):
    NQTR = T // QT // 2
    out = np.empty((B, T, N_EMBD), dtype=np.float32)
    for c in range(N_CORES):
        blk = results[c]["out"]  # [B, NQTR, 128, N_EMBD]
        for u in range(B):
            for q in range(NQTR):
                out[u, q * 1024 + c * 128:q * 1024 + (c + 1) * 128, :] = blk[u, q]
    return out


_NC_CACHE = {}


def kernel(x, W_attn, b_attn, W_proj, b_proj):
    from concourse.bass_utils import run_bass_kernel_spmd
    x = np.asarray(x, dtype=np.float32)
    W_attn = np.asarray(W_attn, dtype=np.float32)
    b_attn = np.asarray(b_attn, dtype=np.float32)
    W_proj = np.asarray(W_proj, dtype=np.float32)
    b_proj = np.asarray(b_proj, dtype=np.float32)
    T = x.shape[1]
    if T not in _NC_CACHE:
        _NC_CACHE[T] = build_nc(T)
    nc = _NC_CACHE[T]
    in_maps = make_inputs(x, W_attn, b_attn, W_proj, b_proj, T)
    res = run_bass_kernel_spmd(nc, in_maps, core_ids=list(range(N_CORES)))
    return assemble(res.results, T)
